# revision 1
# baseline (speedup 1.0000x reference)
"""Trainium2 Bass kernel for nn_LIFLayer (T=512, B=64, C_IN=C_OUT=512).

Strategy (data-parallel over batch, 8 batches/core, no collectives):

The reference per-step recurrence is restructured as:
  G_t   = x_t @ Wx^T + b                      (batched matmul, precomputed)
  S_t   = slow_{t-1} @ Ws^T + G_t             (sequential; 4 fp32r matmuls + I8 matmul)
  sig   = Sigmoid(S_t)                        (ACT)
  d_t   = 0.995^(0.9*sig+0.05) == Square(sc*sig + bs) + Delta   (exact to fp32, ACT)
  slow_t = d_t * slow_{t-1} + x_t             (DVE stt + add, channel-major)
  -- deferred, batched --
  fast_t = 0.9 fast_{t-1} + x_t               (native tensor_tensor_scan)
  z_t   = 2 x_t + fast_t + slow_t             (DVE; 0.5 and 0.1 folded into W01)
  cur_t = z_t @ (0.05 W)                      (batched fp32r matmul)
  v'    = 0.9 v + cur_t ; ns = (v' <= 1) ; acc += ns ; v = v' * ns
  out   = 1 - acc / T

Channel-major state layout [128c x (4k,8b)]; sigmoid output is transposed
back to channel-major with 4 tiny PE transposes per step.
"""

import math
import numpy as np

T, B, C = 512, 64, 512
CO = 512
NCORES = 8
BL = B // NCORES  # 8 batches per core
ALPHA = 0.9
A_FAST = 0.9
A_SLOW = 0.995

# quadratic expansion of d = A_SLOW**(0.9*sig + 0.05) = a0 + a1*sig + a2*sig^2
_L = math.log(A_SLOW)
_a0 = 1.0 + 0.05 * _L + 0.00125 * _L * _L
_a1 = 0.9 * _L + 0.045 * _L * _L
_a2 = 0.405 * _L * _L
SC = math.sqrt(_a2)
BS = _a1 / (2.0 * SC)
DELTA = _a0 - BS * BS

_NC_CACHE = {}


def build_nc(t_steps=T):
    import concourse.bass as bass
    import concourse.bacc as bacc
    import concourse.mybir as mybir
    from concourse.tile import TileContext
    from contextlib import ExitStack

    f32 = mybir.dt.float32
    f32r = mybir.dt.float32r
    AF = mybir.ActivationFunctionType
    OP = mybir.AluOpType

    NCH = t_steps // 16          # 16-tick chunks
    NDEF = t_steps // 64         # 64-tick deferred chunks

    nc = bacc.Bacc()

    seq_l = nc.dram_tensor("seq_l", [t_steps, BL, C], f32, kind="ExternalInput")
    wsT_d = nc.dram_tensor("wsT", [C, C], f32r, kind="ExternalInput")
    wxT_d = nc.dram_tensor("wxT", [C, C], f32r, kind="ExternalInput")
    w01_d = nc.dram_tensor("w01", [C, CO], f32r, kind="ExternalInput")
    bias_d = nc.dram_tensor("biasv", [1, C], f32r, kind="ExternalInput")
    eye8_d = nc.dram_tensor("eye8", [8, 8], f32r, kind="ExternalInput")
    eye128_d = nc.dram_tensor("eye128", [128, 128], f32, kind="ExternalInput")
    ones_d = nc.dram_tensor("ones1", [1, 128], f32r, kind="ExternalInput")
    out_d = nc.dram_tensor("out_l", [BL, CO], f32, kind="ExternalOutput")

    def r(ap):  # matmul operands are already float32r-typed
        return ap

    with TileContext(nc) as tc, ExitStack() as ctx:
        dram = ctx.enter_context(tc.tile_pool(name="dram", bufs=1, space="DRAM"))
        xt_dram = dram.tile([128, 4, BL, t_steps], f32r)   # [c-part][k][b][t]
        slow_dram = dram.tile([128, 4, BL, t_steps], f32r)
        g_dram = dram.tile([NCH, 128, CO], f32r)     # [chunk][(16t,8b)][c]

        consts = ctx.enter_context(tc.tile_pool(name="consts", bufs=1))
        wsT_sb = consts.tile([128, 4, C], f32r)
        wxT_sb = consts.tile([128, 4, C], f32r)
        w01_sb = consts.tile([128, 4, CO], f32r)
        bias_sb = consts.tile([1, C], f32r)
        eye8_sb = consts.tile([8, 8], f32r)
        eye128_sb = consts.tile([128, 128], f32)
        ones_sb = consts.tile([1, 128], f32r)
        zeros32 = consts.tile([128, 4, 8], f32)
        c09 = consts.tile([128, 64], f32)
        bs_ap = consts.tile([128, 1], f32)
        nc.vector.memset(bs_ap, BS)

        nc.sync.dma_start(wsT_sb, wsT_d.rearrange("(k p) j -> p k j", p=128))
        nc.sync.dma_start(wxT_sb, wxT_d.rearrange("(k p) j -> p k j", p=128))
        nc.sync.dma_start(w01_sb, w01_d.rearrange("(k p) j -> p k j", p=128))
        nc.sync.dma_start(bias_sb, bias_d[:, :])
        nc.sync.dma_start(eye8_sb, eye8_d[:, :])
        nc.sync.dma_start(eye128_sb, eye128_d[:, :])
        nc.sync.dma_start(ones_sb, ones_d[:, :])
        nc.vector.memset(zeros32, 0.0)
        nc.vector.memset(c09, A_FAST)

        # ---------------- phase 1: gate recurrence (fused G precompute) -----
        small = ctx.enter_context(tc.tile_pool(name="small", bufs=2))
        phase1 = ExitStack()
        gate = phase1.enter_context(tc.tile_pool(name="gate", bufs=3))
        gpsum = phase1.enter_context(tc.tile_pool(name="gpsum", bufs=2, space="PSUM"))
        gstage = phase1.enter_context(tc.tile_pool(name="gstage", bufs=8))

        prev_slow = None  # AP of slowT(t-1)  [128,(4,8)] view provider
        for u in range(NCH):
            # --- chunk prep: load seq, transpose to channel-major, G matmul
            seqc = gate.tile([128, C], f32, tag="seqc")
            nc.sync.dma_start(
                seqc, seq_l[u * 16:(u + 1) * 16].rearrange("t b c -> (t b) c")
            )
            xt_ps = gpsum.tile([128, 4, BL, 16], f32, tag="xt_ps")
            for k in range(4):
                nc.tensor.transpose(
                    xt_ps[:, k].rearrange("p b t -> p t b"),
                    seqc[:, k * 128:(k + 1) * 128],
                    eye128_sb,
                )
            xt_c = gate.tile([128, 4, BL, 16], f32r, tag="xt_c")
            nc.scalar.activation(
                xt_c.rearrange("p k b t -> p (k b t)"),
                xt_ps.rearrange("p k b t -> p (k b t)"),
                AF.Copy,
            )
            nc.sync.dma_start(xt_dram[:, :, :, 16 * u:16 * (u + 1)], xt_c)

            g_ps = gpsum.tile([128, CO], f32, tag="g_ps")
            for k in range(4):
                nc.tensor.matmul(
                    g_ps,
                    r(xt_c[:, k].rearrange("p b t -> p (b t)")),
                    r(wxT_sb[:, k, :]),
                    start=(k == 0),
                    stop=False,
                )
            nc.tensor.matmul(g_ps, r(ones_sb), r(bias_sb), start=False, stop=True)
            g_c = gate.tile([128, CO], f32r, tag="g_c")
            nc.scalar.activation(g_c, g_ps, AF.Copy)
            nc.sync.dma_start(g_dram[u], g_c)

            slow_c = gate.tile([128, 4, BL, 16], f32r, tag="slow_c")

            # --- 16 sequential ticks
            for j in range(16):
                tau = u * 16 + j
                S = gpsum.tile([8, 512], f32, tag="S")
                if tau > 0:
                    for k in range(4):
                        nc.tensor.matmul(
                            S,
                            r(prev_slow[:, k]),
                            r(wsT_sb[:, k, :]),
                            start=(k == 0),
                            stop=False,
                        )
                gt = gstage.tile([8, 512], f32r, tag="gt")
                nc.sync.dma_start(
                    gt, g_dram[u].rearrange("(b t) c -> t b c", t=16)[j]
                )
                nc.tensor.matmul(
                    S,
                    r(eye8_sb),
                    r(gt),
                    start=(tau == 0),
                    stop=True,
                )
                sig = small.tile([8, 512], f32, tag="sig")
                nc.scalar.activation(sig, S, AF.Sigmoid)
                sigT = gpsum.tile([128, 4, 8], f32, tag="sigT")
                for k in range(4):
                    nc.tensor.transpose(
                        sigT[:, k, :], sig[:, k * 128:(k + 1) * 128],
                        eye128_sb[0:8, 0:8],
                    )
                q = small.tile([128, 4, 8], f32, tag="q")
                nc.scalar.activation(
                    q[:, :, :],
                    sigT[:, :, :],
                    AF.Square,
                    bias=bs_ap,
                    scale=SC,
                )
                p_t = small.tile([128, 4, 8], f32, tag="p_t")
                nc.vector.scalar_tensor_tensor(
                    p_t[:, :, :],
                    q[:, :, :],
                    DELTA,
                    (prev_slow if tau > 0 else zeros32[:, :, :]),
                    op0=OP.add,
                    op1=OP.mult,
                )
                nc.vector.tensor_tensor(
                    slow_c[:, :, :, j],
                    p_t[:, :, :],
                    xt_c[:, :, :, j],
                    op=OP.add,
                )
                prev_slow = slow_c[:, :, :, j]

            nc.sync.dma_start(slow_dram[:, :, :, 16 * u:16 * (u + 1)], slow_c)

        # ---------------- phase 2: deferred fast/z/cur/v ---------------------
        phase1.close()
        dpool = ctx.enter_context(tc.tile_pool(name="dpool", bufs=2))
        dpsum = ctx.enter_context(tc.tile_pool(name="dpsum", bufs=1, space="PSUM"))
        state = ctx.enter_context(tc.tile_pool(name="state", bufs=1))

        v_st = state.tile([128, 4, 8], f32)
        acc = state.tile([128, 4, 8], f32)
        nc.vector.memset(v_st, 0.0)
        nc.vector.memset(acc, 0.0)

        prev_fast = None
        for w in range(NDEF):
            xt64 = dpool.tile([128, 4, BL, 64], f32r, tag="xt64")
            slow64 = dpool.tile([128, 4, BL, 64], f32r, tag="slow64")
            nc.sync.dma_start(xt64, xt_dram[:, :, :, 64 * w:64 * (w + 1)])
            nc.sync.dma_start(slow64, slow_dram[:, :, :, 64 * w:64 * (w + 1)])
            fast64 = dpool.tile([128, 4, BL, 64], f32, tag="fast64")
            for k in range(4):
                for b in range(BL):
                    nc.vector.tensor_tensor_scan(
                        fast64[:, k, b, :],
                        c09,
                        xt64[:, k, b, :],
                        initial=(
                            0.0 if w == 0 else prev_fast[:, k, b, 63:64]
                        ),
                        op0=OP.mult,
                        op1=OP.add,
                    )
            # z = 2x + fast + slow   (in place over xt64)
            zf = xt64.rearrange("p k b t -> p (k b t)")
            nc.vector.scalar_tensor_tensor(
                zf, zf, 2.0, fast64.rearrange("p k b t -> p (k b t)"),
                op0=OP.mult, op1=OP.add,
            )
            nc.gpsimd.tensor_tensor(
                zf, zf, slow64.rearrange("p k b t -> p (k b t)"), op=OP.add
            )
            cur_ps = dpsum.tile([128, 4, BL, 64], f32, tag="cur")
            for m in range(4):
                for k in range(4):
                    nc.tensor.matmul(
                        cur_ps[:, m].rearrange("p b t -> p (b t)"),
                        r(w01_sb[:, k, m * 128:(m + 1) * 128]),
                        r(xt64[:, k].rearrange("p b t -> p (b t)")),
                        start=(k == 0),
                        stop=(k == 3),
                    )
            vp64 = dpool.tile([128, 4, 8, 64], f32, tag="vp64")
            for t in range(64):
                nc.vector.scalar_tensor_tensor(
                    vp64[:, :, :, t],
                    v_st[:, :, :],
                    ALPHA,
                    cur_ps[:, :, :, t],
                    op0=OP.mult,
                    op1=OP.add,
                )
                nc.vector.scalar_tensor_tensor(
                    v_st[:, :, :],
                    vp64[:, :, :, t],
                    1.0,
                    vp64[:, :, :, t],
                    op0=OP.is_le,
                    op1=OP.mult,
                )
            # batched spike counting for the whole chunk
            ns64 = dpool.tile([128, 4, 8, 64], f32, tag="ns64")
            nc.vector.tensor_scalar(
                ns64.rearrange("p m b t -> p (m b t)"),
                vp64.rearrange("p m b t -> p (m b t)"),
                1.0,
                None,
                op0=OP.is_le,
            )
            nsum = small.tile([128, 4, 8], f32, tag="nsum")
            nc.vector.tensor_reduce(
                nsum[:, :, :], ns64[:, :, :, :], axis=mybir.AxisListType.X,
                op=OP.add,
            )
            nc.vector.tensor_tensor(
                acc[:, :, :], acc[:, :, :], nsum[:, :, :], op=OP.add
            )
            prev_fast = fast64

        res = state.tile([128, 4, 8], f32)
        nc.vector.tensor_scalar(
            res.rearrange("p m b -> p (m b)"),
            acc.rearrange("p m b -> p (m b)"),
            -1.0 / t_steps,
            1.0,
            op0=OP.mult,
            op1=OP.add,
        )
        resT_ps = dpsum.tile([8, 4, 128], f32)
        for m in range(4):
            nc.tensor.transpose(resT_ps[:, m, :], res[:, m, :], eye128_sb)
        resT = state.tile([8, 4, 128], f32)
        nc.scalar.activation(
            resT.rearrange("b m p -> b (m p)"),
            resT_ps.rearrange("b m p -> b (m p)"),
            AF.Copy,
        )
        nc.sync.dma_start(out_d[:, :], resT.rearrange("b m p -> b (m p)"))

    nc.finalize()
    return nc


def _prep_shared(seq, W, ctrl_w, ctrl_b):
    f = np.float32
    wsT = np.ascontiguousarray(ctrl_w[:, C:].T, dtype=f)
    wxT = np.ascontiguousarray(ctrl_w[:, :C].T, dtype=f)
    w01 = np.ascontiguousarray((1.0 - ALPHA) * 0.5 * W, dtype=f)
    bias = np.ascontiguousarray(ctrl_b[None, :], dtype=f)
    eye8 = np.eye(8, dtype=f)
    eye128 = np.eye(128, dtype=f)
    ones1 = np.ones((1, 128), dtype=f)
    return dict(wsT=wsT, wxT=wxT, w01=w01, biasv=bias, eye8=eye8,
                eye128=eye128, ones1=ones1)


LAST_EXEC_NS = None


def kernel(seq, W, ctrl_w, ctrl_b):
    global LAST_EXEC_NS
    import os
    from concourse.bass_utils import run_bass_kernel_spmd

    seq = np.asarray(seq, dtype=np.float32)
    t_steps = seq.shape[0]
    if t_steps not in _NC_CACHE:
        _NC_CACHE[t_steps] = build_nc(t_steps)
    nc = _NC_CACHE[t_steps]

    shared = _prep_shared(seq, np.asarray(W), np.asarray(ctrl_w),
                          np.asarray(ctrl_b))
    in_maps = []
    for c in range(NCORES):
        m = dict(shared)
        m["seq_l"] = np.ascontiguousarray(seq[:, c * BL:(c + 1) * BL, :])
        in_maps.append(m)

    trace = bool(os.environ.get("KERNEL_TRACE"))
    results = run_bass_kernel_spmd(
        nc, in_maps, core_ids=list(range(NCORES)), trace=trace
    )
    LAST_EXEC_NS = results.exec_time_ns
    return np.concatenate([res["out_l"] for res in results.results], axis=0)


if __name__ == "__main__":
    import reference

    inputs = {k: np.asarray(v) for k, v in reference.setup_inputs().items()}
    out = kernel(**inputs)
    print("kernel output", out.shape, out.dtype, out.mean())



# revision 18
# speedup vs baseline: 3.0493x; 3.0493x over previous
"""Trainium2 Bass kernel for nn_LIFLayer (T=512, B=64, C_IN=C_OUT=512).

Strategy: data-parallel over batch (8 batches/core, no collectives), with
both sequential recurrences parallelized over time:

1. slow recurrence  (slow_t = d_t * slow_{t-1} + x_t,  d_t nonlinear in
   slow_{t-1}) via global Picard/DEER iteration: 4 rounds of
     S = slow_prev @ WsT + G          (batched fp32r matmul, all t at once)
     sigma = Sigmoid(S)               (ACT, batched)
     d = (SC*sigma+BS)^2 + DELTA      (exact quadratic of A_SLOW**warp)
     slow = linscan(d, x)             (DVE tensor_tensor_scan, frozen d)
   Numpy-validated: out maxerr 0.0039 (tolerance 2e-2).

2. v/spike recurrence via segmented-exact evaluation: v resets to exactly 0
   on spikes (~38% rate), so 8 time-segments run in parallel in the free
   dim of wide DVE/Pool ops; each has a 48-step warmup from v=0 which
   reconverges to the exact trajectory (P(miss) ~ 1e-10 per chain).

Everything else (G = x@WxT + b, fast scan, z = 2x+fast+slow, cur = z@.05W,
spike counting) is batched and overlapped across PE/ACT/DVE/Pool.

Channel-major state layout [128 chan-part, 4 k, T, 8 b] (t-major so that
16tx8b matmul-stationary slices flatten to one contiguous free dim).
"""

import math
import numpy as np

T, B, C, CO = 512, 64, 512, 512
NCORES = 8
BL = B // NCORES
ALPHA = 0.9
A_FAST = 0.9
A_SLOW = 0.995
N_ITER = 4
TSEG = 64
WARM = 48

# quadratic expansion of d = A_SLOW**(0.9*sig + 0.05) = a0 + a1*sig + a2*sig^2
_L = math.log(A_SLOW)
_a0 = 1.0 + 0.05 * _L + 0.00125 * _L * _L
_a1 = 0.9 * _L + 0.045 * _L * _L
_a2 = 0.405 * _L * _L
SC = math.sqrt(_a2)
BS = _a1 / (2.0 * SC)
DELTA = _a0 - BS * BS
DMID = A_SLOW ** 0.5

_NC_CACHE = {}


def build_nc(t_steps=T):
    import concourse.bass as bass
    import concourse.bacc as bacc
    import concourse.mybir as mybir
    from concourse.tile import TileContext
    from contextlib import ExitStack

    f32 = mybir.dt.float32
    f32r = mybir.dt.float32r
    f16 = mybir.dt.float16
    AF = mybir.ActivationFunctionType
    OP = mybir.AluOpType

    NBLK = t_steps // 16            # (t,b)-blocks of 128 rows (16t x 8b)
    KSEG = max(1, t_steps // TSEG)  # v-loop segments
    warm = WARM if KSEG > 1 else 0

    nc = bacc.Bacc()

    seq_l = nc.dram_tensor("seq_l", [t_steps, BL, C], f32, kind="ExternalInput")
    wsT_d = nc.dram_tensor("wsT", [C, C], f32r, kind="ExternalInput")
    wxT_d = nc.dram_tensor("wxTh", [C, C], f16, kind="ExternalInput")
    w01_d = nc.dram_tensor("w01", [C, CO], f32r, kind="ExternalInput")
    bias_d = nc.dram_tensor("biasvh", [1, C], f16, kind="ExternalInput")
    ones_d = nc.dram_tensor("ones1h", [1, 128], f16, kind="ExternalInput")
    eyef_d = nc.dram_tensor("eye128f", [128, 128], f32, kind="ExternalInput")
    eyeh_d = nc.dram_tensor("eye128h", [128, 128], f16, kind="ExternalInput")
    out_d = nc.dram_tensor("out_l", [BL, CO], f32, kind="ExternalOutput")

    with TileContext(nc) as tc, ExitStack() as ctx:
        consts = ctx.enter_context(tc.tile_pool(name="consts", bufs=1))
        eyef_sb = consts.tile([128, 128], f32)
        eyeh_sb = consts.tile([128, 128], f16)
        bs_ap = consts.tile([128, 1], f32)
        neg1_ap = consts.tile([128, 1], f32)
        dconst = consts.tile([128, t_steps], f32)
        delta_c = consts.tile([128, t_steps], f32)
        nc.sync.dma_start(eyef_sb, eyef_d[:, :])
        nc.sync.dma_start(eyeh_sb, eyeh_d[:, :])
        nc.vector.memset(bs_ap, BS)
        nc.vector.memset(neg1_ap, -1.0)
        nc.vector.memset(dconst, DMID)
        nc.vector.memset(delta_c, DELTA)

        # slow holds the slow traj, then z in the tail (right-side stack)
        slowstack = ExitStack()
        slowpool = slowstack.enter_context(
            tc.tile_pool(name="slowpool", bufs=1, side="right"))
        slow = slowpool.tile([128, 4, t_steps + 1, BL], f32r)  # slow[t=0]=0
        for _k in range(4):
            nc.vector.memset(slow[:, _k, 0, :].bitcast(f32), 0.0)

        # x16 lives until the z-assembly in the tail (right side, above slow)
        xstack = ExitStack()
        xpool = xstack.enter_context(
            tc.tile_pool(name="xpool", bufs=1, side="right"))
        x16 = xpool.tile([128, 4, t_steps, BL], f16)       # channel-major x

        # phase-1/2 tensors (freed before tail)
        ph12 = ExitStack()
        iw = ph12.enter_context(tc.tile_pool(name="iw", bufs=1))
        wsT_sb = iw.tile([128, 4, C], f32r)
        wxT_sb = iw.tile([128, 4, C], f16)
        bias_sb = iw.tile([1, C], f16)
        ones_sb = iw.tile([1, 128], f16)
        g_sb = iw.tile([128, NBLK, C], f16)
        sgT = iw.tile([128, 4, t_steps, BL], f16)
        nc.sync.dma_start(wsT_sb, wsT_d.rearrange("(k p) j -> p k j", p=128))
        nc.sync.dma_start(wxT_sb, wxT_d.rearrange("(k p) j -> p k j", p=128))
        nc.sync.dma_start(bias_sb, bias_d[:, :])
        nc.sync.dma_start(ones_sb, ones_d[:, :])

        sstage = ph12.enter_context(tc.tile_pool(name="sstage", bufs=3))
        qstage = ph12.enter_context(tc.tile_pool(name="qstage", bufs=2))
        ipsum = ph12.enter_context(tc.tile_pool(name="ipsum", bufs=3, space="PSUM"))
        tpsum = ph12.enter_context(tc.tile_pool(name="tpsum", bufs=2, space="PSUM"))

        # ---------------- setup: transpose x, compute G --------------------
        for u in range(NBLK):
            seqc = sstage.tile([128, C], f32, tag="seqc")
            nc.sync.dma_start(
                seqc, seq_l[u * 16:(u + 1) * 16].rearrange("t b c -> (t b) c")
            )
            xt_ps = tpsum.tile([128, 4, 128], f32, tag="t_ps")
            for k in range(4):
                nc.tensor.transpose(
                    xt_ps[:, k, :], seqc[:, k * 128:(k + 1) * 128], eyef_sb
                )
            sl = slice(16 * u, 16 * (u + 1))
            nc.scalar.activation(
                x16[:, :, sl, :],
                xt_ps.rearrange("p k (t b) -> p k t b", t=16),
                AF.Copy,
            )
            g_ps = ipsum.tile([128, C], f32, tag="mm_ps")
            for k in range(4):
                nc.tensor.matmul(
                    g_ps,
                    x16[:, k, sl, :].rearrange("p t b -> p (t b)"),
                    wxT_sb[:, k, :],
                    start=(k == 0),
                    stop=False,
                )
            nc.tensor.matmul(g_ps, ones_sb, bias_sb, start=False, stop=True)
            nc.vector.tensor_copy(g_sb[:, u, :], g_ps)

        # ---------------- scan0: slow with constant d ----------------------
        for k in range(4):
            for b in range(BL):
                nc.vector.tensor_tensor_scan(
                    slow[:, k, 1:t_steps + 1, b],
                    dconst,
                    x16[:, k, :, b],
                    initial=0.0,
                    op0=OP.mult,
                    op1=OP.add,
                )

        # ---------------- Picard iterations --------------------------------
        for it in range(N_ITER):
            for u in range(NBLK):
                sl = slice(16 * u, 16 * (u + 1))
                s_ps = ipsum.tile([128, C], f32, tag="mm_ps")
                for k in range(4):
                    nc.tensor.matmul(
                        s_ps,
                        slow[:, k, sl, :].rearrange("p t b -> p (t b)"),
                        wsT_sb[:, k, :],
                        start=(k == 0),
                        stop=False,
                    )
                nc.tensor.matmul(s_ps, eyeh_sb, g_sb[:, u, :],
                                 start=False, stop=True)
                sig32 = sstage.tile([128, C], f32, tag="sig32")
                nc.scalar.activation(sig32, s_ps, AF.Sigmoid)
                sgT_ps = tpsum.tile([128, 4, 128], f32, tag="t_ps")
                for k in range(4):
                    nc.tensor.transpose(
                        sgT_ps[:, k, :], sig32[:, k * 128:(k + 1) * 128], eyef_sb
                    )
                nc.vector.tensor_copy(
                    sgT[:, :, sl, :],
                    sgT_ps.rearrange("p k (t b) -> p k t b", t=16),
                )
            for k in range(4):
                for b in range(BL):
                    q = qstage.tile([128, t_steps], f32, tag="q")
                    nc.scalar.activation(
                        q, sgT[:, k, :, b], AF.Square, bias=bs_ap, scale=SC
                    )
                    dd = qstage.tile([128, t_steps], f32, tag="dd")
                    nc.gpsimd.tensor_tensor(dd, q, delta_c, op=OP.add)
                    nc.vector.tensor_tensor_scan(
                        slow[:, k, 1:t_steps + 1, b],
                        dd,
                        x16[:, k, :, b],
                        initial=0.0,
                        op0=OP.mult,
                        op1=OP.add,
                    )

        # ---------------- tail A: fast, z, cur -----------------------------
        ph12.close()
        tailA = ExitStack()
        tw = tailA.enter_context(tc.tile_pool(name="tw", bufs=1))
        w01_sb = tw.tile([128, 4, CO], f32r)
        nc.sync.dma_start(w01_sb, w01_d.rearrange("(k p) j -> p k j", p=128))
        cstage = tailA.enter_context(tc.tile_pool(name="cstage", bufs=2))
        cpsum = tailA.enter_context(tc.tile_pool(name="cpsum", bufs=2, space="PSUM"))
        ctpsum = tailA.enter_context(
            tc.tile_pool(name="ctpsum", bufs=2, space="PSUM"))
        faststack = ExitStack()
        fastpool = faststack.enter_context(tc.tile_pool(name="fastpool", bufs=1))
        fast = fastpool.tile([128, 4, t_steps, BL], f32r)

        nc.vector.memset(dconst, A_FAST)  # reuse as fast-scan coefficient
        for k in range(4):
            for b in range(BL):
                nc.vector.tensor_tensor_scan(
                    fast[:, k, :, b],
                    dconst,
                    x16[:, k, :, b],
                    initial=0.0,
                    op0=OP.mult,
                    op1=OP.add,
                )
        # z = 2x + fast + slow (in place over fast), seg-major for cur overlap
        nseg = max(1, t_steps // TSEG)
        zseg = t_steps // nseg
        for s in range(nseg):
            zsl = slice(zseg * s, zseg * (s + 1))
            for k in range(4):
                zslow = slow[:, k, 1 + zseg * s:1 + zseg * (s + 1), :]
                if k < 2:
                    nc.vector.scalar_tensor_tensor(
                        fast[:, k, zsl, :], x16[:, k, zsl, :], 2.0,
                        fast[:, k, zsl, :], op0=OP.mult, op1=OP.add,
                    )
                    nc.vector.tensor_tensor(
                        zslow, fast[:, k, zsl, :], zslow, op=OP.add)
                else:
                    nc.gpsimd.tensor_tensor(
                        fast[:, k, zsl, :], fast[:, k, zsl, :],
                        x16[:, k, zsl, :], op=OP.add,
                    )
                    nc.gpsimd.tensor_tensor(
                        fast[:, k, zsl, :], fast[:, k, zsl, :],
                        x16[:, k, zsl, :], op=OP.add,
                    )
                    nc.gpsimd.tensor_tensor(
                        zslow, fast[:, k, zsl, :], zslow, op=OP.add)
        xstack.close()   # x16 dead
        faststack.close()  # fast dead (z now lives in slow's buffer)

        # curT: channel-major cur with a 64-col zero head (uniform v-loop)
        PADT = 64 + t_steps
        ctstack = ExitStack()
        ctpool = ctstack.enter_context(tc.tile_pool(name="ctpool", bufs=1))
        curT = ctpool.tile([128, PADT, 4, BL], f32)
        nc.vector.memset(curT[:, 0:64, :, :].rearrange("p t m b -> p (t m b)"),
                         0.0)
        for u in range(NBLK):
            sl = slice(16 * u, 16 * (u + 1))
            cur_ps = cpsum.tile([128, CO], f32, tag="cur_ps")
            for k in range(4):
                nc.tensor.matmul(
                    cur_ps,
                    slow[:, k, 1 + 16 * u:1 + 16 * (u + 1), :].rearrange(
                        "p t b -> p (t b)"),
                    w01_sb[:, k, :],
                    start=(k == 0),
                    stop=(k == 3),
                )
            cur32 = cstage.tile([128, CO], f32, tag="cur32")
            nc.scalar.activation(cur32, cur_ps, AF.Copy)
            curT_ps = ctpsum.tile([128, 4, 128], f32, tag="curT_ps")
            for m in range(4):
                nc.tensor.transpose(
                    curT_ps[:, m, :], cur32[:, m * 128:(m + 1) * 128], eyef_sb
                )
            nc.vector.tensor_copy(
                curT[:, 64 + 16 * u:64 + 16 * (u + 1), :, :],
                curT_ps.rearrange("p m (t b) -> p t m b", t=16),
            )
        slowstack.close()  # z dead

        # ---------------- tail B: segmented v / spike loop ------------------
        # curT/vp layout [128, 64+T, 4, 8]: seg s, local t -> col 64*s+16+i
        # (i = step idx incl. warmup); head [0:64) is zeros so seg 0's
        # warmup integrates zero drive (exact). Warmup vp writes land in
        # earlier cols later overwritten by the owning segment's real steps.
        tailB = ExitStack()
        vpool = tailB.enter_context(tc.tile_pool(name="vpool", bufs=1))
        vpsum = tailB.enter_context(tc.tile_pool(name="vpsum", bufs=1, space="PSUM"))
        vp = vpool.tile([128, PADT, 4, BL], f32)
        vstate = vpool.tile([128, KSEG, 4, BL], f32)
        nc.vector.memset(vstate, 0.0)

        SCH = PADT // TSEG  # segment chunks incl. the zero head
        cview = curT.rearrange("p (s t) m b -> p s t (m b)", t=TSEG)
        vview = vp.rearrange("p (s t) m b -> p s t (m b)", t=TSEG)
        vs = vstate.rearrange("p s m b -> p s (m b)")
        off0 = TSEG - warm
        for i in range(warm + TSEG):
            col = off0 + i
            s0, tin = (0, col) if col < TSEG else (1, col - TSEG)
            cin = cview[:, s0:s0 + KSEG, tin, :]
            o1 = vview[:, s0:s0 + KSEG, tin, :]
            nc.vector.scalar_tensor_tensor(
                o1, vs, ALPHA, cin, op0=OP.mult, op1=OP.add)
            nc.vector.scalar_tensor_tensor(
                vs, o1, 1.0, o1, op0=OP.is_le, op1=OP.mult)

        # spike count via ACT Sign accumulate: sum_t sign(vp-1) = 2*S - T
        acc = vpool.tile([128, 4, BL], f32)
        sgn_dump = vpool.tile([128, t_steps], f32)
        for m in range(4):
            for b in range(BL):
                nc.scalar.activation(
                    sgn_dump, vp[:, 64:64 + t_steps, m, b], AF.Sign,
                    bias=neg1_ap,
                    accum_out=acc[:, m, b:b + 1],
                )
        res = vpool.tile([128, 4, BL], f32)
        nc.vector.tensor_scalar(
            res.rearrange("p m b -> p (m b)"),
            acc.rearrange("p m b -> p (m b)"),
            0.5 / t_steps, 0.5, op0=OP.mult, op1=OP.add,
        )
        resT_ps = vpsum.tile([8, 4, 128], f32)
        for m in range(4):
            nc.tensor.transpose(resT_ps[:, m, :], res[:, m, :], eyef_sb)
        resT = vpool.tile([8, 4, 128], f32)
        nc.scalar.activation(
            resT.rearrange("b m p -> b (m p)"),
            resT_ps.rearrange("b m p -> b (m p)"),
            AF.Copy,
        )
        nc.sync.dma_start(out_d[:, :], resT.rearrange("b m p -> b (m p)"))
        tailB.close()
        ctstack.close()
        tailA.close()

    nc.finalize()
    return nc


def _prep_shared(W, ctrl_w, ctrl_b):
    f = np.float32
    h = np.float16
    wsT = np.ascontiguousarray(ctrl_w[:, C:].T, dtype=f)
    wxTh = np.ascontiguousarray(ctrl_w[:, :C].T, dtype=h)
    w01 = np.ascontiguousarray((1.0 - ALPHA) * 0.5 * W, dtype=f)
    biasvh = np.ascontiguousarray(ctrl_b[None, :], dtype=h)
    ones1h = np.ones((1, 128), dtype=h)
    eye128f = np.eye(128, dtype=f)
    eye128h = np.eye(128, dtype=h)
    return dict(wsT=wsT, wxTh=wxTh, w01=w01, biasvh=biasvh, ones1h=ones1h,
                eye128f=eye128f, eye128h=eye128h)


LAST_EXEC_NS = None


def kernel(seq, W, ctrl_w, ctrl_b):
    global LAST_EXEC_NS
    import os
    from concourse.bass_utils import run_bass_kernel_spmd

    seq = np.asarray(seq, dtype=np.float32)
    t_steps = seq.shape[0]
    if t_steps not in _NC_CACHE:
        _NC_CACHE[t_steps] = build_nc(t_steps)
    nc = _NC_CACHE[t_steps]

    shared = _prep_shared(np.asarray(W), np.asarray(ctrl_w), np.asarray(ctrl_b))
    in_maps = []
    for c in range(NCORES):
        m = dict(shared)
        m["seq_l"] = np.ascontiguousarray(seq[:, c * BL:(c + 1) * BL, :])
        in_maps.append(m)

    trace = bool(os.environ.get("KERNEL_TRACE"))
    results = run_bass_kernel_spmd(
        nc, in_maps, core_ids=list(range(NCORES)), trace=trace
    )
    LAST_EXEC_NS = results.exec_time_ns
    return np.concatenate([res["out_l"] for res in results.results], axis=0)


if __name__ == "__main__":
    import reference

    inputs = {k: np.asarray(v) for k, v in reference.setup_inputs().items()}
    out = kernel(**inputs)
    print("kernel output", out.shape, out.dtype, out.mean())


# revision 22
# speedup vs baseline: 3.9836x; 1.3064x over previous
"""Trainium2 Bass kernel for nn_LIFLayer (T=512, B=64, C_IN=C_OUT=512).

Strategy: data-parallel over batch (8 batches/core, no collectives), with
both sequential recurrences parallelized over time:

1. slow recurrence  (slow_t = d_t * slow_{t-1} + x_t,  d_t nonlinear in
   slow_{t-1}) via global Picard/DEER iteration: 4 rounds of
     S = slow_prev @ WsT + G          (batched fp32r matmul, all t at once)
     sigma = Sigmoid(S)               (ACT, batched)
     d = (SC*sigma+BS)^2 + DELTA      (exact quadratic of A_SLOW**warp)
     slow = linscan(d, x)             (DVE tensor_tensor_scan, frozen d)
   Numpy-validated: out maxerr 0.0039 (tolerance 2e-2).

2. v/spike recurrence via segmented-exact evaluation: v resets to exactly 0
   on spikes (~38% rate), so 8 time-segments run in parallel in the free
   dim of wide DVE/Pool ops; each has a 48-step warmup from v=0 which
   reconverges to the exact trajectory (P(miss) ~ 1e-10 per chain).

Everything else (G = x@WxT + b, fast scan, z = 2x+fast+slow, cur = z@.05W,
spike counting) is batched and overlapped across PE/ACT/DVE/Pool.

Channel-major state layout [128 chan-part, 4 k, T, 8 b] (t-major so that
16tx8b matmul-stationary slices flatten to one contiguous free dim).
"""

import math
import numpy as np

T, B, C, CO = 512, 64, 512, 512
NCORES = 8
BL = B // NCORES
ALPHA = 0.9
A_FAST = 0.9
A_SLOW = 0.995
N_ITER = 4
TSEG = 32
WARM = 32

# quadratic expansion of d = A_SLOW**(0.9*sig + 0.05) = a0 + a1*sig + a2*sig^2
_L = math.log(A_SLOW)
_a0 = 1.0 + 0.05 * _L + 0.00125 * _L * _L
_a1 = 0.9 * _L + 0.045 * _L * _L
_a2 = 0.405 * _L * _L
SC = math.sqrt(_a2)
BS = _a1 / (2.0 * SC)
DELTA = _a0 - BS * BS
DMID = A_SLOW ** 0.5

_NC_CACHE = {}


def build_nc(t_steps=T):
    import concourse.bass as bass
    import concourse.bacc as bacc
    import concourse.mybir as mybir
    from concourse.tile import TileContext
    from contextlib import ExitStack

    f32 = mybir.dt.float32
    f32r = mybir.dt.float32r
    f16 = mybir.dt.float16
    AF = mybir.ActivationFunctionType
    OP = mybir.AluOpType

    NBLK = t_steps // 16            # (t,b)-blocks of 128 rows (16t x 8b)
    KSEG = max(1, t_steps // TSEG)  # v-loop segments
    warm = WARM if KSEG > 1 else 0

    nc = bacc.Bacc()

    seq_l = nc.dram_tensor("seq_l", [t_steps, BL, C], f32, kind="ExternalInput")
    wsT_d = nc.dram_tensor("wsT", [C, C], f32r, kind="ExternalInput")
    wxT_d = nc.dram_tensor("wxTh", [C, C], f16, kind="ExternalInput")
    w01_d = nc.dram_tensor("w01", [C, CO], f32r, kind="ExternalInput")
    bias_d = nc.dram_tensor("biasvh", [1, C], f16, kind="ExternalInput")
    ones_d = nc.dram_tensor("ones1h", [1, 128], f16, kind="ExternalInput")
    eyef_d = nc.dram_tensor("eye128f", [128, 128], f32, kind="ExternalInput")
    eyeh_d = nc.dram_tensor("eye128h", [128, 128], f16, kind="ExternalInput")
    out_d = nc.dram_tensor("out_l", [BL, CO], f32, kind="ExternalOutput")

    with TileContext(nc) as tc, ExitStack() as ctx:
        consts = ctx.enter_context(tc.tile_pool(name="consts", bufs=1))
        eyef_sb = consts.tile([128, 128], f32)
        eyeh_sb = consts.tile([128, 128], f16)
        bs_ap = consts.tile([128, 1], f32)
        neg1_ap = consts.tile([128, 1], f32)
        dconst = consts.tile([128, t_steps], f32)
        delta_c = consts.tile([128, t_steps], f32)
        nc.sync.dma_start(eyef_sb, eyef_d[:, :])
        nc.sync.dma_start(eyeh_sb, eyeh_d[:, :])
        nc.vector.memset(bs_ap, BS)
        nc.vector.memset(neg1_ap, -1.0)
        nc.vector.memset(dconst, DMID)
        nc.vector.memset(delta_c, DELTA)

        # slow holds the slow traj, then z in the tail (right-side stack)
        slowstack = ExitStack()
        slowpool = slowstack.enter_context(
            tc.tile_pool(name="slowpool", bufs=1, side="right"))
        slow = slowpool.tile([128, 4, t_steps + 1, BL], f32r)  # slow[t=0]=0
        for _k in range(4):
            nc.vector.memset(slow[:, _k, 0, :].bitcast(f32), 0.0)

        # x16 lives until the z-assembly in the tail (right side, above slow)
        xstack = ExitStack()
        xpool = xstack.enter_context(
            tc.tile_pool(name="xpool", bufs=1, side="right"))
        x16 = xpool.tile([128, 4, t_steps, BL], f16)       # channel-major x

        # phase-1/2 tensors (freed before tail)
        ph12 = ExitStack()
        iw = ph12.enter_context(tc.tile_pool(name="iw", bufs=1))
        wsT_sb = iw.tile([128, 4, C], f32r)
        wxT_sb = iw.tile([128, 4, C], f16)
        bias_sb = iw.tile([1, C], f16)
        ones_sb = iw.tile([1, 128], f16)
        g_sb = iw.tile([128, NBLK, C], f16)
        sgT = iw.tile([128, 4, t_steps, BL], f16)
        nc.sync.dma_start(wsT_sb, wsT_d.rearrange("(k p) j -> p k j", p=128))
        nc.sync.dma_start(wxT_sb, wxT_d.rearrange("(k p) j -> p k j", p=128))
        nc.sync.dma_start(bias_sb, bias_d[:, :])
        nc.sync.dma_start(ones_sb, ones_d[:, :])

        sstage = ph12.enter_context(tc.tile_pool(name="sstage", bufs=3))
        qstage = ph12.enter_context(tc.tile_pool(name="qstage", bufs=2))
        ipsum = ph12.enter_context(tc.tile_pool(name="ipsum", bufs=4, space="PSUM"))
        tpsum = ph12.enter_context(tc.tile_pool(name="tpsum", bufs=2, space="PSUM"))

        # ---------------- setup: transpose x, compute G --------------------
        for u in range(NBLK):
            seqc = sstage.tile([128, C], f32, tag="seqc")
            nc.sync.dma_start(
                seqc, seq_l[u * 16:(u + 1) * 16].rearrange("t b c -> (t b) c")
            )
            xt_ps = tpsum.tile([128, 4, 128], f32, tag="t_ps")
            for k in range(4):
                nc.tensor.transpose(
                    xt_ps[:, k, :], seqc[:, k * 128:(k + 1) * 128], eyef_sb
                )
            sl = slice(16 * u, 16 * (u + 1))
            nc.scalar.activation(
                x16[:, :, sl, :],
                xt_ps.rearrange("p k (t b) -> p k t b", t=16),
                AF.Copy,
            )
            g_ps = ipsum.tile([128, C], f32, tag="mm_ps")
            for k in range(4):
                nc.tensor.matmul(
                    g_ps,
                    x16[:, k, sl, :].rearrange("p t b -> p (t b)"),
                    wxT_sb[:, k, :],
                    start=(k == 0),
                    stop=False,
                )
            nc.tensor.matmul(g_ps, ones_sb, bias_sb, start=False, stop=True)
            nc.vector.tensor_copy(g_sb[:, u, :], g_ps)

        # ---------------- scan0: slow with constant d ----------------------
        ch0 = max(1, t_steps // 2)
        for c in range(t_steps // ch0):
            off = c * ch0
            for k in range(4):
                for b in range(BL):
                    nc.vector.tensor_tensor_scan(
                        slow[:, k, 1 + off:1 + off + ch0, b],
                        dconst[:, 0:ch0],
                        x16[:, k, off:off + ch0, b],
                        initial=slow[:, k, off:off + 1, b],
                        op0=OP.mult,
                        op1=OP.add,
                    )

        # ---------------- Picard iterations --------------------------------
        for it in range(N_ITER):
            for u in range(NBLK):
                sl = slice(16 * u, 16 * (u + 1))
                s_ps = ipsum.tile([128, C], f32, tag="mm_ps")
                for k in range(4):
                    nc.tensor.matmul(
                        s_ps,
                        slow[:, k, sl, :].rearrange("p t b -> p (t b)"),
                        wsT_sb[:, k, :],
                        start=(k == 0),
                        stop=False,
                    )
                nc.tensor.matmul(s_ps, eyeh_sb, g_sb[:, u, :],
                                 start=False, stop=True)
                sig16 = sstage.tile([128, C], f16, tag="sig16")
                nc.scalar.activation(sig16, s_ps, AF.Sigmoid)
                sgT_ps = tpsum.tile([128, 4, 128], f16, tag="th_ps")
                for k in range(4):
                    nc.tensor.transpose(
                        sgT_ps[:, k, :], sig16[:, k * 128:(k + 1) * 128], eyeh_sb
                    )
                nc.vector.tensor_copy(
                    sgT[:, :, sl, :],
                    sgT_ps.rearrange("p k (t b) -> p k t b", t=16),
                )
            ch = max(1, t_steps // 2)
            for c in range(t_steps // ch):
                off = c * ch
                for k in range(4):
                    for b in range(BL):
                        q = qstage.tile([128, ch], f32, tag="q")
                        nc.scalar.activation(
                            q, sgT[:, k, off:off + ch, b], AF.Square,
                            bias=bs_ap, scale=SC,
                        )
                        dd = qstage.tile([128, ch], f32, tag="dd")
                        nc.gpsimd.tensor_tensor(
                            dd, q, delta_c[:, 0:ch], op=OP.add)
                        nc.vector.tensor_tensor_scan(
                            slow[:, k, 1 + off:1 + off + ch, b],
                            dd,
                            x16[:, k, off:off + ch, b],
                            initial=slow[:, k, off:off + 1, b],
                            op0=OP.mult,
                            op1=OP.add,
                        )

        # ------- tail A: fast/z/cur pipelined by t-quarters -----------------
        ph12.close()
        tailA = ExitStack()
        tw = tailA.enter_context(tc.tile_pool(name="tw", bufs=1))
        w01_sb = tw.tile([128, 4, CO], f32r)
        nc.sync.dma_start(w01_sb, w01_d.rearrange("(k p) j -> p k j", p=128))
        cstage = tailA.enter_context(tc.tile_pool(name="cstage", bufs=2))
        cpsum = tailA.enter_context(tc.tile_pool(name="cpsum", bufs=2, space="PSUM"))
        ctpsum = tailA.enter_context(
            tc.tile_pool(name="ctpsum", bufs=2, space="PSUM"))
        faststack = ExitStack()
        fastpool = faststack.enter_context(tc.tile_pool(name="fastpool", bufs=1))
        QT = max(16, t_steps // 4)            # fast/z quarter length
        NQ = t_steps // QT
        fastc = fastpool.tile([128, 4, QT, BL], f32r)
        fb = fastpool.tile([128, 4, 1, BL], f32)  # fast boundary carry

        # curT: channel-major cur with a TSEG-col zero head (uniform v-loop)
        PADT = TSEG + t_steps
        ctstack = ExitStack()
        ctpool = ctstack.enter_context(tc.tile_pool(name="ctpool", bufs=1))
        curT = ctpool.tile([128, PADT, 4, BL], f32)
        nc.vector.memset(
            curT[:, 0:TSEG, :, :].rearrange("p t m b -> p (t m b)"), 0.0)

        nc.vector.memset(dconst, A_FAST)  # reuse as fast-scan coefficient
        for qc in range(NQ):
            toff = QT * qc
            # fast for this quarter (carry via fb)
            for k in range(4):
                for b in range(BL):
                    nc.vector.tensor_tensor_scan(
                        fastc[:, k, :, b],
                        dconst[:, 0:QT],
                        x16[:, k, toff:toff + QT, b],
                        initial=(0.0 if qc == 0 else fb[:, k, 0:1, b]),
                        op0=OP.mult,
                        op1=OP.add,
                    )
            if qc < NQ - 1:
                nc.vector.tensor_copy(fb, fastc[:, :, QT - 1:QT, :])
            # z = 2x + fast + slow, in place into slow
            for k in range(4):
                zslow = slow[:, k, 1 + toff:1 + toff + QT, :]
                nc.vector.scalar_tensor_tensor(
                    fastc[:, k, :, :], x16[:, k, toff:toff + QT, :], 2.0,
                    fastc[:, k, :, :], op0=OP.mult, op1=OP.add,
                )
                nc.vector.tensor_tensor(
                    zslow, fastc[:, k, :, :], zslow, op=OP.add)
            # cur blocks for this quarter
            for u in range(qc * NBLK // NQ, (qc + 1) * NBLK // NQ):
                cur_ps = cpsum.tile([128, CO], f32, tag="cur_ps")
                for k in range(4):
                    nc.tensor.matmul(
                        cur_ps,
                        slow[:, k, 1 + 16 * u:1 + 16 * (u + 1), :].rearrange(
                            "p t b -> p (t b)"),
                        w01_sb[:, k, :],
                        start=(k == 0),
                        stop=(k == 3),
                    )
                cur32 = cstage.tile([128, CO], f32, tag="cur32")
                nc.scalar.activation(cur32, cur_ps, AF.Copy)
                curT_ps = ctpsum.tile([128, 4, 128], f32, tag="curT_ps")
                for m in range(4):
                    nc.tensor.transpose(
                        curT_ps[:, m, :], cur32[:, m * 128:(m + 1) * 128],
                        eyef_sb,
                    )
                nc.vector.tensor_copy(
                    curT[:, TSEG + 16 * u:TSEG + 16 * (u + 1), :, :],
                    curT_ps.rearrange("p m (t b) -> p t m b", t=16),
                )
        xstack.close()     # x16 dead
        slowstack.close()  # z dead

        # ---------------- tail B: segmented v / spike loop ------------------
        # curT/vp layout [128, TSEG+T, 4, 8]: seg s col TSEG*s+off0+i at step
        # i; head [0:TSEG) zeros so seg 0's warmup integrates zero drive
        # (exact). Warmup vp writes land in cols later overwritten by the
        # owning segment's real steps.
        tailB = ExitStack()
        vpool = tailB.enter_context(tc.tile_pool(name="vpool", bufs=1))
        vpsum = tailB.enter_context(tc.tile_pool(name="vpsum", bufs=1, space="PSUM"))
        vp = vpool.tile([128, PADT, 4, BL], f32)
        vstate = vpool.tile([128, KSEG, 4, BL], f32)
        nc.vector.memset(vstate, 0.0)

        SCH = PADT // TSEG  # segment chunks incl. the zero head
        cview = curT.rearrange("p (s t) m b -> p s t (m b)", t=TSEG)
        vview = vp.rearrange("p (s t) m b -> p s t (m b)", t=TSEG)
        vs = vstate.rearrange("p s m b -> p s (m b)")
        off0 = TSEG - warm
        for i in range(warm + TSEG):
            col = off0 + i
            s0, tin = (0, col) if col < TSEG else (1, col - TSEG)
            cin = cview[:, s0:s0 + KSEG, tin, :]
            o1 = vview[:, s0:s0 + KSEG, tin, :]
            nc.vector.scalar_tensor_tensor(
                o1, vs, ALPHA, cin, op0=OP.mult, op1=OP.add)
            nc.vector.scalar_tensor_tensor(
                vs, o1, 1.0, o1, op0=OP.is_le, op1=OP.mult)

        # spike count via ACT Sign accumulate: sum_t sign(vp-1) = 2*S - T
        acc = vpool.tile([128, 4, BL], f32)
        sgn_dump = vpool.tile([128, t_steps], f32)
        for m in range(4):
            for b in range(BL):
                nc.scalar.activation(
                    sgn_dump, vp[:, TSEG:TSEG + t_steps, m, b], AF.Sign,
                    bias=neg1_ap,
                    accum_out=acc[:, m, b:b + 1],
                )
        res = vpool.tile([128, 4, BL], f32)
        nc.vector.tensor_scalar(
            res.rearrange("p m b -> p (m b)"),
            acc.rearrange("p m b -> p (m b)"),
            0.5 / t_steps, 0.5, op0=OP.mult, op1=OP.add,
        )
        resT_ps = vpsum.tile([8, 4, 128], f32)
        for m in range(4):
            nc.tensor.transpose(resT_ps[:, m, :], res[:, m, :], eyef_sb)
        resT = vpool.tile([8, 4, 128], f32)
        nc.scalar.activation(
            resT.rearrange("b m p -> b (m p)"),
            resT_ps.rearrange("b m p -> b (m p)"),
            AF.Copy,
        )
        nc.sync.dma_start(out_d[:, :], resT.rearrange("b m p -> b (m p)"))
        tailB.close()
        ctstack.close()
        faststack.close()
        tailA.close()

    nc.finalize()
    return nc


def _prep_shared(W, ctrl_w, ctrl_b):
    f = np.float32
    h = np.float16
    wsT = np.ascontiguousarray(ctrl_w[:, C:].T, dtype=f)
    wxTh = np.ascontiguousarray(ctrl_w[:, :C].T, dtype=h)
    w01 = np.ascontiguousarray((1.0 - ALPHA) * 0.5 * W, dtype=f)
    biasvh = np.ascontiguousarray(ctrl_b[None, :], dtype=h)
    ones1h = np.ones((1, 128), dtype=h)
    eye128f = np.eye(128, dtype=f)
    eye128h = np.eye(128, dtype=h)
    return dict(wsT=wsT, wxTh=wxTh, w01=w01, biasvh=biasvh, ones1h=ones1h,
                eye128f=eye128f, eye128h=eye128h)


LAST_EXEC_NS = None


def kernel(seq, W, ctrl_w, ctrl_b):
    global LAST_EXEC_NS
    import os
    from concourse.bass_utils import run_bass_kernel_spmd

    seq = np.asarray(seq, dtype=np.float32)
    t_steps = seq.shape[0]
    if t_steps not in _NC_CACHE:
        _NC_CACHE[t_steps] = build_nc(t_steps)
    nc = _NC_CACHE[t_steps]

    shared = _prep_shared(np.asarray(W), np.asarray(ctrl_w), np.asarray(ctrl_b))
    in_maps = []
    for c in range(NCORES):
        m = dict(shared)
        m["seq_l"] = np.ascontiguousarray(seq[:, c * BL:(c + 1) * BL, :])
        in_maps.append(m)

    trace = bool(os.environ.get("KERNEL_TRACE"))
    results = run_bass_kernel_spmd(
        nc, in_maps, core_ids=list(range(NCORES)), trace=trace
    )
    LAST_EXEC_NS = results.exec_time_ns
    return np.concatenate([res["out_l"] for res in results.results], axis=0)


if __name__ == "__main__":
    import reference

    inputs = {k: np.asarray(v) for k, v in reference.setup_inputs().items()}
    out = kernel(**inputs)
    print("kernel output", out.shape, out.dtype, out.mean())
